# revision 36
# baseline (speedup 1.0000x reference)
"""Trainium2 Bass kernel for an autoregressive-flow (MAF) layer.

Reference computation (per region r, batch-network b):
    xr[n, d]   = x[n, region_idx[r, d]]                      # [N, D]
    h1 = relu(xr @ (W1*M1)[r,b])                             # [N, H]
    h2 = relu(h1 @ (W2*M2)[r,b])                             # [N, H]
    o  = h2 @ (W3*M3)[r,b]                                   # [N, 2D]
    shift = o[:, 0::2]; log_scale = o[:, 1::2]
    u  = (xr - shift) * exp(-log_scale)
    ll[n, r, b] = sum_d(-0.5*u^2 - 0.5*log(2*pi) - log_scale)

Sharding: region axis R=8 across the 8 NeuronCores; each core handles its
region's B=16 networks over all N=2048 samples.

Device dataflow (per core, "transposed" orientation):
    - xtb   [128, 2048] bf16: x-slice transposed, replicated on 4 partition
      row-groups (feeds 4x row-packed K=32 matmuls + the identity matmul).
    - Weights/masks arrive bf16 (device would round them to bf16 anyway;
      masks are exact 0/1), packed w||m per layer so each group-of-4-networks
      needs one DMA trigger per layer (~600 ns of descriptor-gen each).
    - h1T/h2T as [H=128, 512] tiles per (b, n-chunk); relu = the PSUM->SBUF
      move, alternating between ScalarE and VectorE. Matmuls bf16; the
      log-likelihood reduction matmuls run fp32r on fp32 tail values.
    - Final layer split into shift / log_scale halves, 4 networks col-packed
      per PSUM bank. (shift - x) is produced directly in PSUM by seeding the
      accumulation with a negated tiled-identity matmul.
    - tail per group: A=(s-x)^2 [ACT], Bexp=exp(-2l) [ACT], u^2=A*Bexp [DVE],
      l copy [DVE]; sum_d reductions are block-ones matmuls (-0.5 / -1 folded
      into the ones weights) accumulated into one [16, 512] PSUM tile per
      n-chunk, then bias + store.
"""

import ml_dtypes
import numpy as np

import concourse.bacc as bacc
import concourse.mybir as mybir
from concourse.bass_utils import run_bass_kernel_spmd
from concourse.tile import TileContext

R, B, D, H, N, F = 8, 16, 32, 128, 2048, 256
HALF_LOG_2PI = 0.9189385332046727
N_CORES = 8
CHUNK = 512
F32 = mybir.dt.float32
F32R = mybir.dt.float32r
BF16 = mybir.dt.bfloat16


def _consts():
    # Negated tiled identity: out[m, n] = -xt[m % 32, n] when used as lhsT
    # against rhs = xt[0:32, :].
    neg_i4 = np.zeros((D, 128), np.float32)
    for m in range(128):
        neg_i4[m % D, m] = -1.0
    # Block-ones reduction weights [128, 4 groups, 16 nets]: for group g,
    # column j = 4g+bp sums partition rows 32bp..32bp+31.
    llw1 = np.zeros((128, 4, 16), np.float32)  # -0.5 blocks (acts on u^2)
    llw2 = np.zeros((128, 4, 16), np.float32)  # -1 blocks (acts on log_scale)
    for g in range(4):
        for bp in range(4):
            llw1[32 * bp : 32 * (bp + 1), g, 4 * g + bp] = -0.5
            llw2[32 * bp : 32 * (bp + 1), g, 4 * g + bp] = -1.0
    return neg_i4, llw1, llw2


def build_nc(n_total=N):
    assert n_total % CHUNK == 0
    n_chunks = n_total // CHUNK

    nc = bacc.Bacc(
        "TRN2",
        target_bir_lowering=False,
        debug=False,
        enable_asserts=False,
        num_devices=N_CORES,
    )

    xt4_d = nc.declare_dram_parameter("xt4", [128, n_total], BF16, isOutput=False)
    wm1_d = nc.declare_dram_parameter("wm1", [128, 2, 4, 128], BF16, isOutput=False)
    wm2_d = nc.declare_dram_parameter("wm2", [128, 2, 16, 128], BF16, isOutput=False)
    wm3_d = nc.declare_dram_parameter(
        "wm3", [128, 2, 16, 2, 32], BF16, isOutput=False
    )
    out_d = nc.declare_dram_parameter("out", [n_chunks, 16, CHUNK], F32, isOutput=True)

    neg_i4_np, llw1_np, llw2_np = _consts()
    neg_i4_d = nc.inline_tensor(neg_i4_np.astype(ml_dtypes.bfloat16), "neg_i4")
    llw_d = nc.inline_tensor(np.stack([llw1_np, llw2_np], axis=1), "llw")

    with TileContext(nc) as tc:
        with (
            tc.tile_pool(name="const", bufs=1) as cpool,
            tc.tile_pool(name="wload", bufs=2) as lpool,
            tc.tile_pool(name="act", bufs=3) as apool,
            tc.tile_pool(name="tail", bufs=3) as tpool,
            tc.tile_pool(name="p1", bufs=2, space="PSUM") as p1pool,
            tc.tile_pool(name="p2", bufs=2, space="PSUM") as p2pool,
            tc.tile_pool(name="pt", bufs=2, space="PSUM") as ptpool,
            tc.tile_pool(name="pl", bufs=1, space="PSUM") as plpool,
            tc.tile_pool(name="pll", bufs=1, space="PSUM") as pllpool,
        ):
            xtb = cpool.tile([128, n_total], BF16, tag="xtb")
            neg_i4 = cpool.tile([D, 128], BF16, tag="negi4")
            llw = cpool.tile([128, 2, 4, 16], F32R, tag="llw")
            nc.sync.dma_start(out=xtb[:], in_=xt4_d[:])
            nc.sync.dma_start(out=neg_i4[:], in_=neg_i4_d[:])
            llwstage = lpool.tile([128, 2, 4, 16], F32, tag="llwf")
            nc.sync.dma_start(out=llwstage[:], in_=llw_d[:])
            nc.vector.tensor_copy(out=llw[:], in_=llwstage[:])
            llw1 = llw[:, 0]
            llw2 = llw[:, 1]

            # Masked weights, computed once and kept resident. DMA + mask
            # multiplies are split per group-of-4-networks so chunk-0 compute
            # can start while later groups' weights are still in flight.
            w1m = cpool.tile([128, 4, 128], BF16, tag="w1m")
            w2m = cpool.tile([128, 16, 128], BF16, tag="w2m")
            w3m = cpool.tile([128, 16, 2, 32], BF16, tag="w3m")
            for g in range(4):
                bs = slice(4 * g, 4 * (g + 1))
                wm1raw = lpool.tile([128, 2, 128], BF16, tag="l1")
                nc.sync.dma_start(out=wm1raw[:], in_=wm1_d[:, :, g, :])
                nc.vector.tensor_mul(
                    out=w1m[:, g, :], in0=wm1raw[:, 0], in1=wm1raw[:, 1]
                )
                wm2raw = lpool.tile([128, 2, 4, 128], BF16, tag="l2")
                nc.sync.dma_start(out=wm2raw[:], in_=wm2_d[:, :, bs, :])
                nc.vector.tensor_mul(
                    out=w2m[:, bs, :], in0=wm2raw[:, 0], in1=wm2raw[:, 1]
                )
                wm3raw = lpool.tile([128, 2, 4, 2, 32], BF16, tag="l3")
                nc.sync.dma_start(out=wm3raw[:], in_=wm3_d[:, :, bs, :, :])
                nc.vector.tensor_mul(
                    out=w3m[:, bs, :, :], in0=wm3raw[:, 0], in1=wm3raw[:, 1]
                )

            for c in range(n_chunks):
                cs = slice(c * CHUNK, (c + 1) * CHUNK)
                llps = pllpool.tile([16, CHUNK], F32, tag="llps")
                for g in range(4):
                    s2s = []
                    for bp in range(4):
                        b = 4 * g + bp
                        prow = slice(32 * bp, 32 * (bp + 1))
                        p1 = p1pool.tile([128, CHUNK], F32, tag="p1")
                        nc.tensor.matmul(
                            p1[:],
                            w1m[prow, g, :],
                            xtb[prow, cs],
                            start=True,
                            stop=True,
                            tile_position=(32 * bp, 0),
                        )
                        s1 = apool.tile([128, CHUNK], BF16, tag="s1")
                        if bp % 2 == 0:
                            nc.scalar.activation(
                                s1[:], p1[:], mybir.ActivationFunctionType.Relu
                            )
                        else:
                            nc.vector.tensor_scalar_max(s1[:], p1[:], 0.0)
                        p2 = p2pool.tile([128, CHUNK], F32, tag="p2")
                        nc.tensor.matmul(
                            p2[:], w2m[:, b, :], s1[:], start=True, stop=True
                        )
                        s2 = apool.tile([128, CHUNK], BF16, tag="s2")
                        if bp % 2 == 1:
                            nc.scalar.activation(
                                s2[:], p2[:], mybir.ActivationFunctionType.Relu
                            )
                        else:
                            nc.vector.tensor_scalar_max(s2[:], p2[:], 0.0)
                        s2s.append(s2)

                    # T = shift - x accumulated in PSUM: seed with -x via the
                    # negated tiled identity, then add the shift-half matmuls
                    # (4 networks col-packed).
                    tps = ptpool.tile([128, CHUNK], F32, tag="tps")
                    nc.tensor.matmul(
                        tps[:],
                        neg_i4[:],
                        xtb[0:D, cs],
                        start=True,
                        stop=False,
                        skip_group_check=True,
                        tile_position=(0, 0),
                    )
                    for bp in range(4):
                        b = 4 * g + bp
                        prow = slice(32 * bp, 32 * (bp + 1))
                        nc.tensor.matmul(
                            tps[prow, :],
                            w3m[:, b, 0, :],
                            s2s[bp][:],
                            start=False,
                            stop=(bp == 3),
                            skip_group_check=True,
                            tile_position=(0, 32 * bp),
                        )
                    # log_scale half, 4 networks col-packed into one bank.
                    lps = plpool.tile([128, CHUNK], F32, tag="lps")
                    for bp in range(4):
                        b = 4 * g + bp
                        prow = slice(32 * bp, 32 * (bp + 1))
                        nc.tensor.matmul(
                            lps[prow, :],
                            w3m[:, b, 1, :],
                            s2s[bp][:],
                            start=True,
                            stop=True,
                            tile_position=(0, 32 * bp),
                        )

                    # tail: u^2 = (shift-x)^2 * exp(-2*log_scale)
                    a_sb = tpool.tile([128, CHUNK], F32, tag="a")
                    nc.scalar.activation(
                        a_sb[:], tps[:], mybir.ActivationFunctionType.Square
                    )
                    b_sb = tpool.tile([128, CHUNK], F32, tag="b")
                    nc.scalar.activation(
                        b_sb[:], lps[:], mybir.ActivationFunctionType.Exp, scale=-2.0
                    )
                    l_sb = tpool.tile([128, CHUNK], F32R, tag="l")
                    nc.vector.tensor_copy(out=l_sb[:], in_=lps[:])
                    c_sb = tpool.tile([128, CHUNK], F32R, tag="c")
                    nc.vector.tensor_mul(out=c_sb[:], in0=a_sb[:], in1=b_sb[:])

                    # ll accumulation: -0.5*sum_d(u^2) - sum_d(log_scale)
                    nc.tensor.matmul(
                        llps[:],
                        llw1[:, g, :],
                        c_sb[:],
                        start=(g == 0),
                        stop=False,
                        skip_group_check=True,
                    )
                    nc.tensor.matmul(
                        llps[:],
                        llw2[:, g, :],
                        l_sb[:],
                        start=False,
                        stop=(g == 3),
                        skip_group_check=True,
                    )

                ll_sb = tpool.tile([16, CHUNK], F32, tag="ll")
                nc.vector.tensor_scalar_add(
                    ll_sb[:], llps[:], float(-D * HALF_LOG_2PI)
                )
                nc.sync.dma_start(out=out_d[c], in_=ll_sb[:])

    nc.compile()
    return nc


def shard_inputs(x, W1, W2, W3, M1, M2, M3, region_idx, n_total=N):
    """Per-core input dicts: pure gather/transpose/replicate layout prep."""
    x = np.asarray(x, dtype=np.float32)
    region_idx = np.asarray(region_idx)
    in_maps = []
    for r in range(N_CORES):
        xr = x[:n_total, region_idx[r]]  # [n, D]
        xt = np.ascontiguousarray(xr.T)  # [D, n]
        xt4 = np.ascontiguousarray(np.tile(xt, (4, 1)))  # [128, n]

        def prep1(w):
            w = np.asarray(w[r], dtype=np.float32)  # [16, 32, 128]
            return np.ascontiguousarray(
                w.reshape(4, 4, D, H).transpose(1, 2, 0, 3).reshape(128, 4, H)
            ).astype(ml_dtypes.bfloat16)

        def prep2(w):
            w = np.asarray(w[r], dtype=np.float32)  # [16, 128, 128]
            return np.ascontiguousarray(w.transpose(1, 0, 2)).astype(
                ml_dtypes.bfloat16
            )

        def prep3(w):
            w = np.asarray(w[r], dtype=np.float32)  # [16, 128, 64]
            return np.ascontiguousarray(
                w.reshape(B, H, D, 2).transpose(1, 0, 3, 2)
            ).astype(ml_dtypes.bfloat16)

        in_maps.append(
            {
                "xt4": xt4.astype(ml_dtypes.bfloat16),
                "wm1": np.ascontiguousarray(
                    np.stack([prep1(W1), prep1(M1)], axis=1)
                ),
                "wm2": np.ascontiguousarray(
                    np.stack([prep2(W2), prep2(M2)], axis=1)
                ),
                "wm3": np.ascontiguousarray(
                    np.stack([prep3(W3), prep3(M3)], axis=1)
                ),
            }
        )
    return in_maps


_NC_CACHE = {}


def run(x, W1, W2, W3, M1, M2, M3, region_idx, trace=False, n_total=N):
    if n_total not in _NC_CACHE:
        _NC_CACHE[n_total] = build_nc(n_total)
    nc = _NC_CACHE[n_total]
    in_maps = shard_inputs(x, W1, W2, W3, M1, M2, M3, region_idx, n_total)
    res = run_bass_kernel_spmd(
        nc, in_maps, core_ids=list(range(N_CORES)), trace=trace
    )
    out = np.empty((n_total, R, B), dtype=np.float32)
    for r in range(N_CORES):
        o = res.results[r]["out"]  # [n_chunks, 16, CHUNK]
        out[:, r, :] = o.transpose(0, 2, 1).reshape(n_total, B)
    return out, res


def kernel(x, W1, W2, W3, M1, M2, M3, region_idx):
    out, _ = run(x, W1, W2, W3, M1, M2, M3, region_idx)
    return out


# revision 39
# speedup vs baseline: 1.0720x; 1.0720x over previous
"""Trainium2 Bass kernel for an autoregressive-flow (MAF) layer.

Reference computation (per region r, batch-network b):
    xr[n, d]   = x[n, region_idx[r, d]]                      # [N, D]
    h1 = relu(xr @ (W1*M1)[r,b])                             # [N, H]
    h2 = relu(h1 @ (W2*M2)[r,b])                             # [N, H]
    o  = h2 @ (W3*M3)[r,b]                                   # [N, 2D]
    shift = o[:, 0::2]; log_scale = o[:, 1::2]
    u  = (xr - shift) * exp(-log_scale)
    ll[n, r, b] = sum_d(-0.5*u^2 - 0.5*log(2*pi) - log_scale)

Sharding: region axis R=8 across the 8 NeuronCores; each core handles its
region's B=16 networks over all N=2048 samples.

Device dataflow (per core, "transposed" orientation):
    - xtb   [128, 2048] bf16: x-slice transposed, replicated on 4 partition
      row-groups (feeds 4x row-packed K=32 matmuls + the identity matmul).
    - Weights/masks arrive bf16 (device would round them to bf16 anyway;
      masks are exact 0/1), packed w||m per layer so each group-of-4-networks
      needs one DMA trigger per layer (~600 ns of descriptor-gen each).
    - h1T/h2T as [H=128, 512] tiles per (b, n-chunk); relu = the PSUM->SBUF
      move, alternating between ScalarE and VectorE. Matmuls bf16; the
      log-likelihood reduction matmuls run fp32r on fp32 tail values.
    - Final layer split into shift / log_scale halves, 4 networks col-packed
      per PSUM bank. (shift - x) is produced directly in PSUM by seeding the
      accumulation with a negated tiled-identity matmul.
    - tail per group: A=(s-x)^2 [ACT], Bexp=exp(-2l) [ACT], u^2=A*Bexp [DVE],
      l copy [DVE]; sum_d reductions are block-ones matmuls (-0.5 / -1 folded
      into the ones weights) accumulated into one [16, 512] PSUM tile per
      n-chunk, then bias + store.
"""

import ml_dtypes
import numpy as np

import concourse.bacc as bacc
import concourse.mybir as mybir
from concourse.bass_utils import run_bass_kernel_spmd
from concourse.tile import TileContext

R, B, D, H, N, F = 8, 16, 32, 128, 2048, 256
HALF_LOG_2PI = 0.9189385332046727
N_CORES = 8
CHUNK = 512
F32 = mybir.dt.float32
F32R = mybir.dt.float32r
BF16 = mybir.dt.bfloat16


def _consts():
    # Negated tiled identity: out[m, n] = -xt[m % 32, n] when used as lhsT
    # against rhs = xt[0:32, :].
    neg_i4 = np.zeros((D, 128), np.float32)
    for m in range(128):
        neg_i4[m % D, m] = -1.0
    # Block-ones reduction weights [128, 4 groups, 16 nets]: for group g,
    # column j = 4g+bp sums partition rows 32bp..32bp+31.
    llw1 = np.zeros((128, 4, 16), np.float32)  # -0.5 blocks (acts on u^2)
    llw2 = np.zeros((128, 4, 16), np.float32)  # -1 blocks (acts on log_scale)
    for g in range(4):
        for bp in range(4):
            llw1[32 * bp : 32 * (bp + 1), g, 4 * g + bp] = -0.5
            llw2[32 * bp : 32 * (bp + 1), g, 4 * g + bp] = -1.0
    return neg_i4, llw1, llw2


def build_nc(n_total=N):
    assert n_total % CHUNK == 0
    n_chunks = n_total // CHUNK

    nc = bacc.Bacc(
        "TRN2",
        target_bir_lowering=False,
        debug=False,
        enable_asserts=False,
        num_devices=N_CORES,
    )

    xt4_d = nc.declare_dram_parameter("xt4", [128, n_total], BF16, isOutput=False)
    wm1_d = nc.declare_dram_parameter("wm1", [128, 2, 4, 128], BF16, isOutput=False)
    wm2_d = nc.declare_dram_parameter("wm2", [128, 2, 16, 128], BF16, isOutput=False)
    wm3_d = nc.declare_dram_parameter(
        "wm3", [128, 2, 16, 2, 32], BF16, isOutput=False
    )
    out_d = nc.declare_dram_parameter("out", [n_chunks, 16, CHUNK], F32, isOutput=True)

    neg_i4_np, llw1_np, llw2_np = _consts()
    neg_i4_d = nc.inline_tensor(neg_i4_np.astype(ml_dtypes.bfloat16), "neg_i4")
    llw_d = nc.inline_tensor(np.stack([llw1_np, llw2_np], axis=1), "llw")

    with TileContext(nc) as tc:
        with (
            tc.tile_pool(name="const", bufs=1) as cpool,
            tc.tile_pool(name="wload", bufs=2) as lpool,
            tc.tile_pool(name="act", bufs=3) as apool,
            tc.tile_pool(name="tail", bufs=3) as tpool,
            tc.tile_pool(name="p1", bufs=2, space="PSUM") as p1pool,
            tc.tile_pool(name="p2", bufs=2, space="PSUM") as p2pool,
            tc.tile_pool(name="pt", bufs=2, space="PSUM") as ptpool,
            tc.tile_pool(name="pl", bufs=1, space="PSUM") as plpool,
            tc.tile_pool(name="pll", bufs=1, space="PSUM") as pllpool,
        ):
            xtb = cpool.tile([128, n_total], BF16, tag="xtb")
            neg_i4 = cpool.tile([D, 128], BF16, tag="negi4")
            llw = cpool.tile([128, 2, 4, 16], F32R, tag="llw")
            nc.sync.dma_start(out=xtb[:], in_=xt4_d[:])
            nc.sync.dma_start(out=neg_i4[:], in_=neg_i4_d[:])
            llwstage = lpool.tile([128, 2, 4, 16], F32, tag="llwf")
            nc.sync.dma_start(out=llwstage[:], in_=llw_d[:])
            nc.vector.tensor_copy(out=llw[:], in_=llwstage[:])
            llw1 = llw[:, 0]
            llw2 = llw[:, 1]

            # Masked weights, computed once and kept resident. DMA + mask
            # multiplies are split per group-of-4-networks so chunk-0 compute
            # can start while later groups' weights are still in flight.
            w1m = cpool.tile([128, 4, 128], BF16, tag="w1m")
            w2m = cpool.tile([128, 16, 128], BF16, tag="w2m")
            w3m = cpool.tile([128, 16, 2, 32], BF16, tag="w3m")
            for g in range(4):
                bs = slice(4 * g, 4 * (g + 1))
                wm1raw = lpool.tile([128, 2, 128], BF16, tag="l1")
                nc.sync.dma_start(out=wm1raw[:], in_=wm1_d[:, :, g, :])
                nc.vector.tensor_mul(
                    out=w1m[:, g, :], in0=wm1raw[:, 0], in1=wm1raw[:, 1]
                )
                wm2raw = lpool.tile([128, 2, 4, 128], BF16, tag="l2")
                nc.sync.dma_start(out=wm2raw[:], in_=wm2_d[:, :, bs, :])
                nc.vector.tensor_mul(
                    out=w2m[:, bs, :], in0=wm2raw[:, 0], in1=wm2raw[:, 1]
                )
                wm3raw = lpool.tile([128, 2, 4, 2, 32], BF16, tag="l3")
                nc.sync.dma_start(out=wm3raw[:], in_=wm3_d[:, :, bs, :, :])
                nc.vector.tensor_mul(
                    out=w3m[:, bs, :, :], in0=wm3raw[:, 0], in1=wm3raw[:, 1]
                )


            for c in range(n_chunks):
                cs = slice(c * CHUNK, (c + 1) * CHUNK)
                llps = pllpool.tile([16, CHUNK], F32, tag="llps")
                for g in range(4):
                    s2s = []
                    for bp in range(4):
                        b = 4 * g + bp
                        prow = slice(32 * bp, 32 * (bp + 1))
                        p1 = p1pool.tile([128, CHUNK], F32, tag="p1")
                        nc.tensor.matmul(
                            p1[:],
                            w1m[prow, g, :],
                            xtb[prow, cs],
                            start=True,
                            stop=True,
                            tile_position=(32 * bp, 0),
                        )
                        s1 = apool.tile([128, CHUNK], BF16, tag="s1")
                        if bp % 2 == 0:
                            nc.scalar.activation(
                                s1[:], p1[:], mybir.ActivationFunctionType.Relu
                            )
                        else:
                            nc.vector.tensor_scalar_max(s1[:], p1[:], 0.0)
                        p2 = p2pool.tile([128, CHUNK], F32, tag="p2")
                        nc.tensor.matmul(
                            p2[:], w2m[:, b, :], s1[:], start=True, stop=True
                        )
                        s2 = apool.tile([128, CHUNK], BF16, tag="s2")
                        if bp % 2 == 1:
                            nc.scalar.activation(
                                s2[:], p2[:], mybir.ActivationFunctionType.Relu
                            )
                        else:
                            nc.vector.tensor_scalar_max(s2[:], p2[:], 0.0)
                        s2s.append(s2)

                    # T = shift - x accumulated in PSUM: seed with -x via the
                    # negated tiled identity, then add the shift-half matmuls
                    # (4 networks col-packed).
                    tps = ptpool.tile([128, CHUNK], F32, tag="tps")
                    nc.tensor.matmul(
                        tps[:],
                        neg_i4[:],
                        xtb[0:D, cs],
                        start=True,
                        stop=False,
                        skip_group_check=True,
                        tile_position=(0, 0),
                    )
                    for bp in range(4):
                        b = 4 * g + bp
                        prow = slice(32 * bp, 32 * (bp + 1))
                        nc.tensor.matmul(
                            tps[prow, :],
                            w3m[:, b, 0, :],
                            s2s[bp][:],
                            start=False,
                            stop=(bp == 3),
                            skip_group_check=True,
                            tile_position=(0, 32 * bp),
                        )
                    # log_scale half, 4 networks col-packed into one bank.
                    lps = plpool.tile([128, CHUNK], F32, tag="lps")
                    for bp in range(4):
                        b = 4 * g + bp
                        prow = slice(32 * bp, 32 * (bp + 1))
                        nc.tensor.matmul(
                            lps[prow, :],
                            w3m[:, b, 1, :],
                            s2s[bp][:],
                            start=True,
                            stop=True,
                            tile_position=(0, 32 * bp),
                        )

                    # tail: u^2 = (shift-x)^2 * exp(-2*log_scale)
                    a_sb = tpool.tile([128, CHUNK], F32, tag="a")
                    nc.scalar.activation(
                        a_sb[:], tps[:], mybir.ActivationFunctionType.Square
                    )
                    b_sb = tpool.tile([128, CHUNK], F32, tag="b")
                    nc.scalar.activation(
                        b_sb[:], lps[:], mybir.ActivationFunctionType.Exp, scale=-2.0
                    )
                    l_sb = tpool.tile([128, CHUNK], F32R, tag="l")
                    nc.vector.tensor_copy(out=l_sb[:], in_=lps[:])
                    c_sb = tpool.tile([128, CHUNK], F32R, tag="c")
                    nc.vector.tensor_mul(out=c_sb[:], in0=a_sb[:], in1=b_sb[:])

                    # ll accumulation: -0.5*sum_d(u^2) - sum_d(log_scale)
                    nc.tensor.matmul(
                        llps[:],
                        llw1[:, g, :],
                        c_sb[:],
                        start=(g == 0),
                        stop=False,
                        skip_group_check=True,
                    )
                    nc.tensor.matmul(
                        llps[:],
                        llw2[:, g, :],
                        l_sb[:],
                        start=False,
                        stop=(g == 3),
                        skip_group_check=True,
                    )

                ll_sb = tpool.tile([16, CHUNK], F32, tag="ll")
                nc.vector.tensor_scalar_add(
                    ll_sb[:], llps[:], float(-D * HALF_LOG_2PI)
                )
                nc.sync.dma_start(out=out_d[c], in_=ll_sb[:])

    nc.compile()
    return nc


def shard_inputs(x, W1, W2, W3, M1, M2, M3, region_idx, n_total=N):
    """Per-core input dicts: pure gather/transpose/replicate layout prep."""
    x = np.asarray(x, dtype=np.float32)
    region_idx = np.asarray(region_idx)
    in_maps = []
    for r in range(N_CORES):
        xr = x[:n_total, region_idx[r]]  # [n, D]
        xt = np.ascontiguousarray(xr.T)  # [D, n]
        xt4 = np.ascontiguousarray(np.tile(xt, (4, 1)))  # [128, n]

        def prep1(w):
            w = np.asarray(w[r], dtype=np.float32)  # [16, 32, 128]
            return np.ascontiguousarray(
                w.reshape(4, 4, D, H).transpose(1, 2, 0, 3).reshape(128, 4, H)
            ).astype(ml_dtypes.bfloat16)

        def prep2(w):
            w = np.asarray(w[r], dtype=np.float32)  # [16, 128, 128]
            return np.ascontiguousarray(w.transpose(1, 0, 2)).astype(
                ml_dtypes.bfloat16
            )

        def prep3(w):
            w = np.asarray(w[r], dtype=np.float32)  # [16, 128, 64]
            return np.ascontiguousarray(
                w.reshape(B, H, D, 2).transpose(1, 0, 3, 2)
            ).astype(ml_dtypes.bfloat16)

        in_maps.append(
            {
                "xt4": xt4.astype(ml_dtypes.bfloat16),
                "wm1": np.ascontiguousarray(
                    np.stack([prep1(W1), prep1(M1)], axis=1)
                ),
                "wm2": np.ascontiguousarray(
                    np.stack([prep2(W2), prep2(M2)], axis=1)
                ),
                "wm3": np.ascontiguousarray(
                    np.stack([prep3(W3), prep3(M3)], axis=1)
                ),
            }
        )
    return in_maps


_NC_CACHE = {}


def run(x, W1, W2, W3, M1, M2, M3, region_idx, trace=False, n_total=N):
    if n_total not in _NC_CACHE:
        _NC_CACHE[n_total] = build_nc(n_total)
    nc = _NC_CACHE[n_total]
    in_maps = shard_inputs(x, W1, W2, W3, M1, M2, M3, region_idx, n_total)
    res = run_bass_kernel_spmd(
        nc, in_maps, core_ids=list(range(N_CORES)), trace=trace
    )
    out = np.empty((n_total, R, B), dtype=np.float32)
    for r in range(N_CORES):
        o = res.results[r]["out"]  # [n_chunks, 16, CHUNK]
        out[:, r, :] = o.transpose(0, 2, 1).reshape(n_total, B)
    return out, res


def kernel(x, W1, W2, W3, M1, M2, M3, region_idx):
    out, _ = run(x, W1, W2, W3, M1, M2, M3, region_idx)
    return out
